# revision 1
# baseline (speedup 1.0000x reference)
"""Gaussian distance loss kernel for 8 Trainium2 NeuronCores.

reference math (per term):
    f[i,j,d] = exp(-0.5*(mu1[i,d]-mu2[j,d])^2 / (v1[i,d]+v2[j,d])) / sqrt(v1+v2)
    term = mean(f)
    out  = vaa + vbb - 2*vab

Sharding: rows i split across 8 cores (128 rows each). Each core holds the
full transposed operand tensors [128(d), 1024(j)] so the per-row values
mu1[i,:], v1[i,:] become per-partition scalar columns — broadcast natively by
tensor_scalar/STT ops and the activation bias port. No broadcast copies.

Per row i (free dim = j, 1024 wide):
    lv = Ln(cv + rv_i)            ACT (bias port does the add)
    t  = Exp(-0.5*lv)             ACT (= rsqrt(v), same table set as Ln)
    y  = (cm - rm_i) * t          DVE scalar_tensor_tensor
    y2 = y*y                      DVE tensor_tensor
    e  = Exp(-0.5*y2)             ACT
    acc[:, col] = sum_j(e*t)      DVE tensor_tensor_reduce
Host sums the [128, 384] partials from all 8 cores in float64.

All inputs are packed into one [128, 4608] array so input staging is a single
DMA (one semaphore) — avoids walrus "too many sync wait commands".
"""

import sys

for _p in ("/opt/trn_rl_repo", "/root/.axon_site/_ro/trn_rl_repo"):
    if _p not in sys.path:
        sys.path.append(_p)

import numpy as np

N = 1024
D = 128
NCORES = 8
ROWS = N // NCORES  # 128 rows per core

# packed input column offsets
O_MA, O_MB, O_LVA, O_LVB = 0, N, 2 * N, 3 * N
O_MAR, O_MBR, O_LVAR, O_LVBR = 4 * N, 4 * N + ROWS, 4 * N + 2 * ROWS, 4 * N + 3 * ROWS
PACKED_W = 4 * N + 4 * ROWS


def build_program(repeat=1):
    import concourse.bacc as bacc
    import concourse.mybir as mybir
    import concourse.tile as tile
    from concourse.alu_op_type import AluOpType

    # The act-table placement pass picks table sets greedily, alternating
    # between the exp-only and ln-only sets here — 385 table reloads (one per
    # row, ~0.5ms each on HW = ~196ms total). All our functions (Exp, Ln,
    # Square) live together in natural_log_exp_and_others; blank the other
    # sets (preserving list indices, which are the act_func_set ids) so the
    # pass must use it: one load for the whole program.
    from concourse import hw_specs as _hw

    _orig_gat = _hw.get_activation_tables.__wrapped__

    def _gat_combined(arch):
        t = dict(_orig_gat(arch))
        return {
            k: (v if k == "natural_log_exp_and_others" else set())
            for k, v in t.items()
        }

    bacc.get_activation_tables = _gat_combined

    f32 = mybir.dt.float32
    Act = mybir.ActivationFunctionType

    nc = bacc.Bacc("TRN2", target_bir_lowering=False, debug=False)

    inp_d = nc.dram_tensor("inp", [D, PACKED_W], f32, kind="ExternalInput")
    acc_out = nc.dram_tensor("acc", [D, 3 * ROWS], f32, kind="ExternalOutput")

    with tile.TileContext(nc) as tc:
        with (
            tc.tile_pool(name="inputs", bufs=1) as inp,
            tc.tile_pool(name="accp", bufs=1) as accp,
            tc.tile_pool(name="lv", bufs=2) as lvp,
            tc.tile_pool(name="t", bufs=3) as tp,
            tc.tile_pool(name="y", bufs=2) as yp,
            tc.tile_pool(name="y2", bufs=2) as y2p,
            tc.tile_pool(name="e", bufs=2) as ep,
            tc.tile_pool(name="scr", bufs=2) as scrp,
        ):
            big = inp.tile([D, PACKED_W], f32, tag="big")
            nc.sync.dma_start(big[:], inp_d[:])

            cm_a = big[:, O_MA : O_MA + N]
            cm_b = big[:, O_MB : O_MB + N]
            rm_a = big[:, O_MAR : O_MAR + ROWS]
            rm_b = big[:, O_MBR : O_MBR + ROWS]

            cv_a = inp.tile([D, N], f32, tag="cv_a")
            cv_b = inp.tile([D, N], f32, tag="cv_b")
            rv_a = inp.tile([D, ROWS], f32, tag="rv_a")
            rv_b = inp.tile([D, ROWS], f32, tag="rv_b")
            nc.scalar.activation(cv_a[:], big[:, O_LVA : O_LVA + N], Act.Exp)
            nc.scalar.activation(cv_b[:], big[:, O_LVB : O_LVB + N], Act.Exp)
            nc.scalar.activation(rv_a[:], big[:, O_LVAR : O_LVAR + ROWS], Act.Exp)
            nc.scalar.activation(rv_b[:], big[:, O_LVBR : O_LVBR + ROWS], Act.Exp)

            acc = accp.tile([D, 3 * ROWS], f32, tag="acc")

            terms = [
                (cm_a, cv_a, rm_a, rv_a),  # vaa
                (cm_b, cv_b, rm_a, rv_a),  # vab
                (cm_b, cv_b, rm_b, rv_b),  # vbb
            ] * repeat

            for ti, (cm, cv, rm, rv) in enumerate(terms):
                for i in range(ROWS):
                    col = (ti % 3) * ROWS + i  # repeats overwrite, same result
                    lv = lvp.tile([D, N], f32, tag="lv")
                    nc.scalar.activation(
                        lv[:], cv[:], Act.Ln, bias=rv[:, i : i + 1], scale=1.0
                    )
                    t = tp.tile([D, N], f32, tag="t")
                    nc.scalar.activation(t[:], lv[:], Act.Exp, scale=-0.5)
                    y = yp.tile([D, N], f32, tag="y")
                    nc.vector.scalar_tensor_tensor(
                        y[:],
                        cm,
                        rm[:, i : i + 1],
                        t[:],
                        AluOpType.subtract,
                        AluOpType.mult,
                    )
                    y2 = y2p.tile([D, N], f32, tag="y2")
                    # y*y on GPSIMD: DVE is the critical engine, GPSIMD is idle
                    nc.gpsimd.tensor_tensor(y2[:], y[:], y[:], AluOpType.mult)
                    e = ep.tile([D, N], f32, tag="e")
                    nc.scalar.activation(e[:], y2[:], Act.Exp, scale=-0.5)
                    scr = scrp.tile([D, N], f32, tag="scr")
                    # tensor_tensor_reduce crashes TRN2 at runtime; STT with
                    # accum_out does the same multiply+sum in one DVE pass.
                    nc.vector.scalar_tensor_tensor(
                        scr[:],
                        e[:],
                        1.0,
                        t[:],
                        AluOpType.mult,
                        AluOpType.mult,
                        accum_out=acc[:, col : col + 1],
                    )

            nc.sync.dma_start(acc_out[:], acc[:])

    nc.compile()
    return nc


_PROGRAM_CACHE = {}


def _get_program(repeat=1):
    if repeat not in _PROGRAM_CACHE:
        _PROGRAM_CACHE[repeat] = build_program(repeat)
    return _PROGRAM_CACHE[repeat]


def pack_inputs(mu_a, logvar_a, mu_b, logvar_b):
    ma_t = np.ascontiguousarray(mu_a.T.astype(np.float32))
    mb_t = np.ascontiguousarray(mu_b.T.astype(np.float32))
    lva_t = np.ascontiguousarray(logvar_a.T.astype(np.float32))
    lvb_t = np.ascontiguousarray(logvar_b.T.astype(np.float32))
    in_maps = []
    for c in range(NCORES):
        r0, r1 = c * ROWS, (c + 1) * ROWS
        packed = np.empty((D, PACKED_W), dtype=np.float32)
        packed[:, O_MA : O_MA + N] = ma_t
        packed[:, O_MB : O_MB + N] = mb_t
        packed[:, O_LVA : O_LVA + N] = lva_t
        packed[:, O_LVB : O_LVB + N] = lvb_t
        packed[:, O_MAR : O_MAR + ROWS] = ma_t[:, r0:r1]
        packed[:, O_MBR : O_MBR + ROWS] = mb_t[:, r0:r1]
        packed[:, O_LVAR : O_LVAR + ROWS] = lva_t[:, r0:r1]
        packed[:, O_LVBR : O_LVBR + ROWS] = lvb_t[:, r0:r1]
        in_maps.append({"inp": packed})
    return in_maps


def run_device(mu_a, logvar_a, mu_b, logvar_b, trace=False, repeat=1):
    from concourse.bass_utils import run_bass_kernel_spmd

    nc = _get_program(repeat)
    in_maps = pack_inputs(mu_a, logvar_a, mu_b, logvar_b)
    return run_bass_kernel_spmd(nc, in_maps, list(range(NCORES)), trace=trace)


def reduce_host(results):
    saa = sab = sbb = 0.0
    for r in results:
        acc = np.asarray(r["acc"], dtype=np.float64)
        saa += acc[:, 0:ROWS].sum()
        sab += acc[:, ROWS : 2 * ROWS].sum()
        sbb += acc[:, 2 * ROWS : 3 * ROWS].sum()
    denom = float(N) * N * D
    return np.float32((saa + sbb - 2.0 * sab) / denom)


def kernel(mu_a, logvar_a, mu_b, logvar_b):
    res = run_device(mu_a, logvar_a, mu_b, logvar_b, trace=False)
    return reduce_host(res.results)



# revision 3
# speedup vs baseline: 13.7367x; 13.7367x over previous
"""Gaussian distance loss kernel for 8 Trainium2 NeuronCores.

reference math (per term):
    f[i,j,d] = exp(-0.5*(mu1[i,d]-mu2[j,d])^2 / vsum) / sqrt(vsum),
    vsum = v1[i,d]+v2[j,d];  out = mean(f_aa) + mean(f_bb) - 2*mean(f_ab).

Identity used: f = exp(-0.5*(dm^2/vsum + ln(vsum))), so the final Exp's
free-dim accumulator (accum_out) performs the j-and-row reduction for
free and no multiply by 1/sqrt(vsum) is needed.

Sharding: rows i split across 8 cores (128 rows each); each core holds
the full transposed operands [128(d), 1024(j)].

This runtime has a large fixed per-instruction cost, engines barely
overlap, and GPSIMD/InstReciprocal are slow. The design therefore uses
the fewest, widest instructions: blocks of B=8 rows are processed by 8
WIDE ops on [128, 8192] bf16 tiles (f32 inputs/accumulator), with
stride-0 broadcast access patterns supplying the per-row operands (no
per-row activations, no broadcast copies):

    DVE : vsum = cv_bc + rv_bc ; dm = cm_bc - rm_bc
    ACT : lv = Ln(vsum) ; rr = Exp(-lv)        (= 1/vsum)
    DVE : u = dm*rr ; w = u*dm (in-place) ; s = w+lv (in-place)
    ACT : e = Exp(-0.5 s) (in-place), accum_out -> acc column

Intermediates are bf16: the per-element rounding errors average out
over the 134M-term mean and the residual bias cancels between the
vaa+vbb and -2*vab terms (measured rel err 5.4e-4 vs 4.8e-4 for f32,
gate 2e-2). The accum_out accumulator and acc tile stay f32. All wide
tiles are single-buffered -- double-buffering and block-grouping both
measured slower on this runtime.
"""

import sys

for _p in ("/opt/trn_rl_repo", "/root/.axon_site/_ro/trn_rl_repo"):
    if _p not in sys.path:
        sys.path.append(_p)

import numpy as np

N = 1024
D = 128
NCORES = 8
ROWS = N // NCORES  # 128 rows per core
B = 8  # rows per wide block
NBLOCKS = ROWS // B

O_MA, O_MB, O_VA, O_VB = 0, N, 2 * N, 3 * N
O_MAR, O_MBR, O_VAR, O_VBR = 4 * N, 4 * N + ROWS, 4 * N + 2 * ROWS, 4 * N + 3 * ROWS
PACKED_W = 4 * N + 4 * ROWS


def build_program(repeat=1):
    import concourse.bacc as bacc
    import concourse.mybir as mybir
    import concourse.tile as tile
    from concourse.alu_op_type import AluOpType
    from concourse import hw_specs as _hw

    # Keep every activation (Ln, Exp) in the one table set that holds them
    # all, so the act-table placement pass emits a single table load
    # instead of one per activation (~0.5ms each on HW).
    _orig_gat = _hw.get_activation_tables.__wrapped__

    def _gat_combined(arch):
        t = dict(_orig_gat(arch))
        return {
            k: (v if k == "natural_log_exp_and_others" else set())
            for k, v in t.items()
        }

    bacc.get_activation_tables = _gat_combined

    f32 = mybir.dt.float32
    bf16 = mybir.dt.bfloat16
    Act = mybir.ActivationFunctionType
    Alu = AluOpType

    WB = B * N

    nc = bacc.Bacc("TRN2", target_bir_lowering=False, debug=False)
    inp_d = nc.dram_tensor("inp", [D, PACKED_W], f32, kind="ExternalInput")
    acc_out = nc.dram_tensor("acc", [D, 3 * NBLOCKS], f32, kind="ExternalOutput")

    def bc_col(t_ap):  # [D, N] column operand -> [D, B, N], stride-0 rows
        return t_ap.unsqueeze(1).broadcast_to((D, B, N))

    def bc_row(t_ap):  # [D, B] row slice -> [D, B, N], stride-0 over j
        return t_ap.unsqueeze(2).broadcast_to((D, B, N))

    def v3(t):  # wide tile [D, WB] -> [D, B, N] view
        return t[:].rearrange("p (b n) -> p b n", b=B)

    with tile.TileContext(nc) as tc:
        with (
            tc.tile_pool(name="inputs", bufs=1) as inp,
            tc.tile_pool(name="accp", bufs=1) as accp,
            tc.tile_pool(name="vsum", bufs=1) as vsump,
            tc.tile_pool(name="lv", bufs=1) as lvp,
            tc.tile_pool(name="rr", bufs=1) as rrp,
            tc.tile_pool(name="dm", bufs=1) as dmp,
            tc.tile_pool(name="u", bufs=1) as up,
        ):
            big = inp.tile([D, PACKED_W], f32, tag="big")
            nc.sync.dma_start(big[:], inp_d[:])

            cm_a = big[:, O_MA : O_MA + N]
            cm_b = big[:, O_MB : O_MB + N]
            cv_a = big[:, O_VA : O_VA + N]
            cv_b = big[:, O_VB : O_VB + N]
            rm_a = big[:, O_MAR : O_MAR + ROWS]
            rm_b = big[:, O_MBR : O_MBR + ROWS]
            rv_a = big[:, O_VAR : O_VAR + ROWS]
            rv_b = big[:, O_VBR : O_VBR + ROWS]

            acc = accp.tile([D, 3 * NBLOCKS], f32, tag="acc")

            terms = [
                (cm_a, cv_a, rm_a, rv_a),  # vaa
                (cm_b, cv_b, rm_a, rv_a),  # vab
                (cm_b, cv_b, rm_b, rv_b),  # vbb
            ] * repeat

            for ti, (cm, cv, rm, rv) in enumerate(terms):
                for blk in range(NBLOCKS):
                    col = (ti % 3) * NBLOCKS + blk  # repeats overwrite
                    i0 = blk * B
                    vsum = vsump.tile([D, WB], bf16, tag="vsum")
                    nc.vector.tensor_tensor(
                        v3(vsum), bc_col(cv), bc_row(rv[:, i0 : i0 + B]), Alu.add
                    )
                    dm = dmp.tile([D, WB], bf16, tag="dm")
                    nc.vector.tensor_tensor(
                        v3(dm), bc_col(cm), bc_row(rm[:, i0 : i0 + B]), Alu.subtract
                    )
                    lv = lvp.tile([D, WB], bf16, tag="lv")
                    nc.scalar.activation(lv[:], vsum[:], Act.Ln)
                    rr = rrp.tile([D, WB], bf16, tag="rr")
                    nc.scalar.activation(rr[:], lv[:], Act.Exp, scale=-1.0)
                    u = up.tile([D, WB], bf16, tag="u")
                    nc.vector.tensor_tensor(u[:], dm[:], rr[:], Alu.mult)
                    nc.vector.tensor_tensor(u[:], u[:], dm[:], Alu.mult)
                    nc.vector.tensor_tensor(u[:], u[:], lv[:], Alu.add)
                    nc.scalar.activation(
                        u[:], u[:], Act.Exp, scale=-0.5,
                        accum_out=acc[:, col : col + 1],
                    )

            nc.sync.dma_start(acc_out[:], acc[:])

    nc.compile()
    return nc


_PROGRAM_CACHE = {}


def _get_program(repeat=1):
    if repeat not in _PROGRAM_CACHE:
        _PROGRAM_CACHE[repeat] = build_program(repeat)
    return _PROGRAM_CACHE[repeat]


def pack_inputs(mu_a, logvar_a, mu_b, logvar_b):
    ma_t = np.ascontiguousarray(np.asarray(mu_a).T.astype(np.float32))
    mb_t = np.ascontiguousarray(np.asarray(mu_b).T.astype(np.float32))
    va_t = np.exp(np.asarray(logvar_a).T.astype(np.float32))
    vb_t = np.exp(np.asarray(logvar_b).T.astype(np.float32))
    in_maps = []
    for c in range(NCORES):
        r0, r1 = c * ROWS, (c + 1) * ROWS
        packed = np.empty((D, PACKED_W), dtype=np.float32)
        packed[:, O_MA : O_MA + N] = ma_t
        packed[:, O_MB : O_MB + N] = mb_t
        packed[:, O_VA : O_VA + N] = va_t
        packed[:, O_VB : O_VB + N] = vb_t
        packed[:, O_MAR : O_MAR + ROWS] = ma_t[:, r0:r1]
        packed[:, O_MBR : O_MBR + ROWS] = mb_t[:, r0:r1]
        packed[:, O_VAR : O_VAR + ROWS] = va_t[:, r0:r1]
        packed[:, O_VBR : O_VBR + ROWS] = vb_t[:, r0:r1]
        in_maps.append({"inp": packed})
    return in_maps


def run_device(mu_a, logvar_a, mu_b, logvar_b, trace=False, repeat=1):
    from concourse.bass_utils import run_bass_kernel_spmd

    nc = _get_program(repeat)
    in_maps = pack_inputs(mu_a, logvar_a, mu_b, logvar_b)
    return run_bass_kernel_spmd(nc, in_maps, list(range(NCORES)), trace=trace)


def reduce_host(results):
    saa = sab = sbb = 0.0
    for r in results:
        acc = np.asarray(r["acc"], dtype=np.float64)
        saa += acc[:, 0:NBLOCKS].sum()
        sab += acc[:, NBLOCKS : 2 * NBLOCKS].sum()
        sbb += acc[:, 2 * NBLOCKS : 3 * NBLOCKS].sum()
    denom = float(N) * N * D
    return np.float32((saa + sbb - 2.0 * sab) / denom)


def kernel(mu_a, logvar_a, mu_b, logvar_b):
    res = run_device(mu_a, logvar_a, mu_b, logvar_b, trace=False)
    return reduce_host(res.results)


# revision 4
# speedup vs baseline: 36.4233x; 2.6515x over previous
"""Gaussian distance loss kernel for 8 Trainium2 NeuronCores.

reference math (per term):
    f[i,j,d] = exp(-0.5*(mu1[i,d]-mu2[j,d])^2 / vsum) / sqrt(vsum),
    vsum = v1[i,d]+v2[j,d];  out = mean(f_aa) + mean(f_bb) - 2*mean(f_ab).

Identity used: f = exp(-0.5*(dm^2/vsum + ln(vsum))), so the final Exp's
free-dim accumulator (accum_out) performs the j-and-row reduction for
free and no multiply by 1/sqrt(vsum) is needed.

Sharding: rows i split across 8 cores (128 rows each); each core holds
the full transposed operands [128(d), 1024(j)].

This runtime has a large fixed per-instruction cost, engines barely
overlap, and GPSIMD/InstReciprocal are slow. The design therefore uses
the fewest, widest instructions: blocks of B=16 rows are processed by 8
WIDE ops on [128, 16384] bf16 tiles (f32 inputs/accumulator), with
stride-0 broadcast access patterns supplying the per-row operands (no
per-row activations, no broadcast copies):

    DVE : vsum = cv_bc + rv_bc ; dm = cm_bc - rm_bc
    ACT : lv = Ln(vsum) ; rr = Exp(-lv)        (= 1/vsum)
    DVE : u = dm*rr ; w = u*dm (in-place) ; s = w+lv (in-place)
    ACT : e = Exp(-0.5 s) (in-place), accum_out -> acc column

Intermediates are bf16: the per-element rounding errors average out
over the 134M-term mean and the residual bias cancels between the
vaa+vbb and -2*vab terms (measured rel err 5.4e-4 vs 4.8e-4 for f32,
gate 2e-2). The accum_out accumulator and acc tile stay f32. All wide
tiles are single-buffered -- double-buffering and block-grouping both
measured slower on this runtime.
"""

import sys

for _p in ("/opt/trn_rl_repo", "/root/.axon_site/_ro/trn_rl_repo"):
    if _p not in sys.path:
        sys.path.append(_p)

import numpy as np

N = 1024
D = 128
NCORES = 8
ROWS = N // NCORES  # 128 rows per core
B = 16  # rows per wide block
NBLOCKS = ROWS // B

O_MA, O_MB, O_VA, O_VB = 0, N, 2 * N, 3 * N
O_MAR, O_MBR, O_VAR, O_VBR = 4 * N, 4 * N + ROWS, 4 * N + 2 * ROWS, 4 * N + 3 * ROWS
PACKED_W = 4 * N + 4 * ROWS


def build_program(repeat=1):
    import concourse.bacc as bacc
    import concourse.mybir as mybir
    import concourse.tile as tile
    from concourse.alu_op_type import AluOpType
    from concourse import hw_specs as _hw

    # Keep every activation (Ln, Exp) in the one table set that holds them
    # all, so the act-table placement pass emits a single table load
    # instead of one per activation (~0.5ms each on HW).
    _orig_gat = _hw.get_activation_tables.__wrapped__

    def _gat_combined(arch):
        t = dict(_orig_gat(arch))
        return {
            k: (v if k == "natural_log_exp_and_others" else set())
            for k, v in t.items()
        }

    bacc.get_activation_tables = _gat_combined

    f32 = mybir.dt.float32
    bf16 = mybir.dt.bfloat16
    Act = mybir.ActivationFunctionType
    Alu = AluOpType

    WB = B * N

    nc = bacc.Bacc("TRN2", target_bir_lowering=False, debug=False)
    inp_d = nc.dram_tensor("inp", [D, PACKED_W], f32, kind="ExternalInput")
    acc_out = nc.dram_tensor("acc", [D, 3 * NBLOCKS], f32, kind="ExternalOutput")

    def bc_col(t_ap):  # [D, N] column operand -> [D, B, N], stride-0 rows
        return t_ap.unsqueeze(1).broadcast_to((D, B, N))

    def bc_row(t_ap):  # [D, B] row slice -> [D, B, N], stride-0 over j
        return t_ap.unsqueeze(2).broadcast_to((D, B, N))

    def v3(t):  # wide tile [D, WB] -> [D, B, N] view
        return t[:].rearrange("p (b n) -> p b n", b=B)

    with tile.TileContext(nc) as tc:
        with (
            tc.tile_pool(name="inputs", bufs=1) as inp,
            tc.tile_pool(name="accp", bufs=1) as accp,
            tc.tile_pool(name="vsum", bufs=1) as vsump,
            tc.tile_pool(name="lv", bufs=1) as lvp,
            tc.tile_pool(name="rr", bufs=1) as rrp,
            tc.tile_pool(name="dm", bufs=1) as dmp,
            tc.tile_pool(name="u", bufs=1) as up,
        ):
            big = inp.tile([D, PACKED_W], f32, tag="big")
            nc.sync.dma_start(big[:], inp_d[:])

            cm_a = big[:, O_MA : O_MA + N]
            cm_b = big[:, O_MB : O_MB + N]
            cv_a = big[:, O_VA : O_VA + N]
            cv_b = big[:, O_VB : O_VB + N]
            rm_a = big[:, O_MAR : O_MAR + ROWS]
            rm_b = big[:, O_MBR : O_MBR + ROWS]
            rv_a = big[:, O_VAR : O_VAR + ROWS]
            rv_b = big[:, O_VBR : O_VBR + ROWS]

            acc = accp.tile([D, 3 * NBLOCKS], f32, tag="acc")

            terms = [
                (cm_a, cv_a, rm_a, rv_a),  # vaa
                (cm_b, cv_b, rm_a, rv_a),  # vab
                (cm_b, cv_b, rm_b, rv_b),  # vbb
            ] * repeat

            for ti, (cm, cv, rm, rv) in enumerate(terms):
                for blk in range(NBLOCKS):
                    col = (ti % 3) * NBLOCKS + blk  # repeats overwrite
                    i0 = blk * B
                    vsum = vsump.tile([D, WB], bf16, tag="vsum")
                    nc.vector.tensor_tensor(
                        v3(vsum), bc_col(cv), bc_row(rv[:, i0 : i0 + B]), Alu.add
                    )
                    dm = dmp.tile([D, WB], bf16, tag="dm")
                    nc.vector.tensor_tensor(
                        v3(dm), bc_col(cm), bc_row(rm[:, i0 : i0 + B]), Alu.subtract
                    )
                    lv = lvp.tile([D, WB], bf16, tag="lv")
                    nc.scalar.activation(lv[:], vsum[:], Act.Ln)
                    rr = rrp.tile([D, WB], bf16, tag="rr")
                    nc.scalar.activation(rr[:], lv[:], Act.Exp, scale=-1.0)
                    u = up.tile([D, WB], bf16, tag="u")
                    nc.vector.tensor_tensor(u[:], dm[:], rr[:], Alu.mult)
                    nc.vector.tensor_tensor(u[:], u[:], dm[:], Alu.mult)
                    nc.vector.tensor_tensor(u[:], u[:], lv[:], Alu.add)
                    nc.scalar.activation(
                        u[:], u[:], Act.Exp, scale=-0.5,
                        accum_out=acc[:, col : col + 1],
                    )

            nc.sync.dma_start(acc_out[:], acc[:])

    nc.compile()
    return nc


_PROGRAM_CACHE = {}


def _get_program(repeat=1):
    if repeat not in _PROGRAM_CACHE:
        _PROGRAM_CACHE[repeat] = build_program(repeat)
    return _PROGRAM_CACHE[repeat]


def pack_inputs(mu_a, logvar_a, mu_b, logvar_b):
    ma_t = np.ascontiguousarray(np.asarray(mu_a).T.astype(np.float32))
    mb_t = np.ascontiguousarray(np.asarray(mu_b).T.astype(np.float32))
    va_t = np.exp(np.asarray(logvar_a).T.astype(np.float32))
    vb_t = np.exp(np.asarray(logvar_b).T.astype(np.float32))
    in_maps = []
    for c in range(NCORES):
        r0, r1 = c * ROWS, (c + 1) * ROWS
        packed = np.empty((D, PACKED_W), dtype=np.float32)
        packed[:, O_MA : O_MA + N] = ma_t
        packed[:, O_MB : O_MB + N] = mb_t
        packed[:, O_VA : O_VA + N] = va_t
        packed[:, O_VB : O_VB + N] = vb_t
        packed[:, O_MAR : O_MAR + ROWS] = ma_t[:, r0:r1]
        packed[:, O_MBR : O_MBR + ROWS] = mb_t[:, r0:r1]
        packed[:, O_VAR : O_VAR + ROWS] = va_t[:, r0:r1]
        packed[:, O_VBR : O_VBR + ROWS] = vb_t[:, r0:r1]
        in_maps.append({"inp": packed})
    return in_maps


def run_device(mu_a, logvar_a, mu_b, logvar_b, trace=False, repeat=1):
    from concourse.bass_utils import run_bass_kernel_spmd

    nc = _get_program(repeat)
    in_maps = pack_inputs(mu_a, logvar_a, mu_b, logvar_b)
    return run_bass_kernel_spmd(nc, in_maps, list(range(NCORES)), trace=trace)


def reduce_host(results):
    saa = sab = sbb = 0.0
    for r in results:
        acc = np.asarray(r["acc"], dtype=np.float64)
        saa += acc[:, 0:NBLOCKS].sum()
        sab += acc[:, NBLOCKS : 2 * NBLOCKS].sum()
        sbb += acc[:, 2 * NBLOCKS : 3 * NBLOCKS].sum()
    denom = float(N) * N * D
    return np.float32((saa + sbb - 2.0 * sab) / denom)


def kernel(mu_a, logvar_a, mu_b, logvar_b):
    res = run_device(mu_a, logvar_a, mu_b, logvar_b, trace=False)
    return reduce_host(res.results)


# revision 5
# speedup vs baseline: 48.6113x; 1.3346x over previous
"""Gaussian distance loss kernel for 8 Trainium2 NeuronCores.

reference math (per term):
    f[i,j,d] = exp(-0.5*(mu1[i,d]-mu2[j,d])^2 / vsum) / sqrt(vsum),
    vsum = v1[i,d]+v2[j,d];  out = mean(f_aa) + mean(f_bb) - 2*mean(f_ab).

Identity used: f = exp(-0.5*(dm^2/vsum + ln(vsum))), so the final Exp's
free-dim accumulator (accum_out) performs the j-and-row reduction for
free and no multiply by 1/sqrt(vsum) is needed.

Sharding: rows i split across 8 cores (128 rows each); each core holds
the full transposed operands [128(d), 1024(j)].

This runtime has a large fixed per-instruction cost, engines barely
overlap, and GPSIMD/InstReciprocal are slow. The design therefore uses
the fewest, widest instructions: blocks of B=32 rows are processed by 8
WIDE ops on [128, 32768] bf16 tiles, using only THREE wide tiles via
in-place chaining (vsum -> lv via in-place Ln; rr carries the
u->w->s->e chain), with stride-0 broadcast access patterns supplying
the per-row operands (no per-row activations, no broadcast copies):

    DVE : vsum = cv_bc + rv_bc ; dm = cm_bc - rm_bc
    ACT : lv = Ln(vsum) in-place ; rr = Exp(-lv)     (= 1/vsum)
    DVE : u = dm*rr ; w = u*dm ; s = w+lv   (all in-place on rr)
    ACT : e = Exp(-0.5 s) (in-place), accum_out -> acc column

Inputs and intermediates are bf16 (accumulator f32): per-element
rounding averages out over the 134M-term mean and the residual bias
cancels between the vaa+vbb and -2*vab terms (measured rel err 1.3e-4,
gate 2e-2). All wide tiles single-buffered -- double-buffering,
block-grouping, and software pipelining all measured slower here.
Measured ~6.8ms/pass vs 179.8ms baseline (~26x).
"""

import sys

for _p in ("/opt/trn_rl_repo", "/root/.axon_site/_ro/trn_rl_repo"):
    if _p not in sys.path:
        sys.path.append(_p)

import ml_dtypes
import numpy as np

N = 1024
D = 128
NCORES = 8
ROWS = N // NCORES  # 128 rows per core
B = 32  # rows per wide block
NBLOCKS = ROWS // B

O_MA, O_MB, O_VA, O_VB = 0, N, 2 * N, 3 * N
O_MAR, O_MBR, O_VAR, O_VBR = 4 * N, 4 * N + ROWS, 4 * N + 2 * ROWS, 4 * N + 3 * ROWS
PACKED_W = 4 * N + 4 * ROWS


def build_program(repeat=1):
    import concourse.bacc as bacc
    import concourse.mybir as mybir
    import concourse.tile as tile
    from concourse.alu_op_type import AluOpType
    from concourse import hw_specs as _hw

    # Keep every activation (Ln, Exp) in the one table set that holds them
    # all, so the act-table placement pass emits a single table load
    # instead of one per activation (~0.5ms each on HW).
    _orig_gat = _hw.get_activation_tables.__wrapped__

    def _gat_combined(arch):
        t = dict(_orig_gat(arch))
        return {
            k: (v if k == "natural_log_exp_and_others" else set())
            for k, v in t.items()
        }

    bacc.get_activation_tables = _gat_combined

    f32 = mybir.dt.float32
    bf16 = mybir.dt.bfloat16
    Act = mybir.ActivationFunctionType
    Alu = AluOpType

    WB = B * N

    nc = bacc.Bacc("TRN2", target_bir_lowering=False, debug=False)
    inp_d = nc.dram_tensor("inp", [D, PACKED_W], bf16, kind="ExternalInput")
    acc_out = nc.dram_tensor("acc", [D, 3 * NBLOCKS], f32, kind="ExternalOutput")

    def bc_col(t_ap):  # [D, N] column operand -> [D, B, N], stride-0 rows
        return t_ap.unsqueeze(1).broadcast_to((D, B, N))

    def bc_row(t_ap):  # [D, B] row slice -> [D, B, N], stride-0 over j
        return t_ap.unsqueeze(2).broadcast_to((D, B, N))

    def v3(t):  # wide tile [D, WB] -> [D, B, N] view
        return t[:].rearrange("p (b n) -> p b n", b=B)

    with tile.TileContext(nc) as tc:
        with (
            tc.tile_pool(name="inputs", bufs=1) as inp,
            tc.tile_pool(name="accp", bufs=1) as accp,
            tc.tile_pool(name="vsum", bufs=1) as vsump,
            tc.tile_pool(name="rr", bufs=1) as rrp,
            tc.tile_pool(name="dm", bufs=1) as dmp,
        ):
            big = inp.tile([D, PACKED_W], bf16, tag="big")
            nc.sync.dma_start(big[:], inp_d[:])

            cm_a = big[:, O_MA : O_MA + N]
            cm_b = big[:, O_MB : O_MB + N]
            cv_a = big[:, O_VA : O_VA + N]
            cv_b = big[:, O_VB : O_VB + N]
            rm_a = big[:, O_MAR : O_MAR + ROWS]
            rm_b = big[:, O_MBR : O_MBR + ROWS]
            rv_a = big[:, O_VAR : O_VAR + ROWS]
            rv_b = big[:, O_VBR : O_VBR + ROWS]

            acc = accp.tile([D, 3 * NBLOCKS], f32, tag="acc")

            terms = [
                (cm_a, cv_a, rm_a, rv_a),  # vaa
                (cm_b, cv_b, rm_a, rv_a),  # vab
                (cm_b, cv_b, rm_b, rv_b),  # vbb
            ] * repeat

            for ti, (cm, cv, rm, rv) in enumerate(terms):
                for blk in range(NBLOCKS):
                    col = (ti % 3) * NBLOCKS + blk  # repeats overwrite
                    i0 = blk * B
                    # 3 wide tiles: vsum (-> lv via in-place Ln),
                    # dm, and rr which carries the whole u->w->s->e chain.
                    vsum = vsump.tile([D, WB], bf16, tag="vsum")
                    nc.vector.tensor_tensor(
                        v3(vsum), bc_col(cv), bc_row(rv[:, i0 : i0 + B]), Alu.add
                    )
                    dm = dmp.tile([D, WB], bf16, tag="dm")
                    nc.vector.tensor_tensor(
                        v3(dm), bc_col(cm), bc_row(rm[:, i0 : i0 + B]), Alu.subtract
                    )
                    nc.scalar.activation(vsum[:], vsum[:], Act.Ln)  # lv
                    rr = rrp.tile([D, WB], bf16, tag="rr")
                    nc.scalar.activation(rr[:], vsum[:], Act.Exp, scale=-1.0)
                    nc.vector.tensor_tensor(rr[:], dm[:], rr[:], Alu.mult)
                    nc.vector.tensor_tensor(rr[:], rr[:], dm[:], Alu.mult)
                    nc.vector.tensor_tensor(rr[:], rr[:], vsum[:], Alu.add)
                    nc.scalar.activation(
                        rr[:], rr[:], Act.Exp, scale=-0.5,
                        accum_out=acc[:, col : col + 1],
                    )

            nc.sync.dma_start(acc_out[:], acc[:])

    nc.compile()
    return nc


_PROGRAM_CACHE = {}


def _get_program(repeat=1):
    if repeat not in _PROGRAM_CACHE:
        _PROGRAM_CACHE[repeat] = build_program(repeat)
    return _PROGRAM_CACHE[repeat]


def pack_inputs(mu_a, logvar_a, mu_b, logvar_b):
    ma_t = np.ascontiguousarray(np.asarray(mu_a).T.astype(np.float32))
    mb_t = np.ascontiguousarray(np.asarray(mu_b).T.astype(np.float32))
    va_t = np.exp(np.asarray(logvar_a).T.astype(np.float32))
    vb_t = np.exp(np.asarray(logvar_b).T.astype(np.float32))
    in_maps = []
    for c in range(NCORES):
        r0, r1 = c * ROWS, (c + 1) * ROWS
        packed = np.empty((D, PACKED_W), dtype=np.float32)  # cast below
        packed[:, O_MA : O_MA + N] = ma_t
        packed[:, O_MB : O_MB + N] = mb_t
        packed[:, O_VA : O_VA + N] = va_t
        packed[:, O_VB : O_VB + N] = vb_t
        packed[:, O_MAR : O_MAR + ROWS] = ma_t[:, r0:r1]
        packed[:, O_MBR : O_MBR + ROWS] = mb_t[:, r0:r1]
        packed[:, O_VAR : O_VAR + ROWS] = va_t[:, r0:r1]
        packed[:, O_VBR : O_VBR + ROWS] = vb_t[:, r0:r1]
        in_maps.append({"inp": packed.astype(ml_dtypes.bfloat16)})
    return in_maps


def run_device(mu_a, logvar_a, mu_b, logvar_b, trace=False, repeat=1):
    from concourse.bass_utils import run_bass_kernel_spmd

    nc = _get_program(repeat)
    in_maps = pack_inputs(mu_a, logvar_a, mu_b, logvar_b)
    return run_bass_kernel_spmd(nc, in_maps, list(range(NCORES)), trace=trace)


def reduce_host(results):
    saa = sab = sbb = 0.0
    for r in results:
        acc = np.asarray(r["acc"], dtype=np.float64)
        saa += acc[:, 0:NBLOCKS].sum()
        sab += acc[:, NBLOCKS : 2 * NBLOCKS].sum()
        sbb += acc[:, 2 * NBLOCKS : 3 * NBLOCKS].sum()
    denom = float(N) * N * D
    return np.float32((saa + sbb - 2.0 * sab) / denom)


def kernel(mu_a, logvar_a, mu_b, logvar_b):
    res = run_device(mu_a, logvar_a, mu_b, logvar_b, trace=False)
    return reduce_host(res.results)
